# revision 60
# baseline (speedup 1.0000x reference)
"""Causal attention kernel for 8 Trainium2 NeuronCores — fp8 DoubleRow edition.

Problem: x[4,2048,1024] fp32, Wq/Wk/Wv[1024,1024] fp32 (nn.Linear: y = x @ W.T),
single-head causal attention, softmax(QK^T/sqrt(D)) @ V.

Sharding: 2 cores per batch; within a batch, queries are split by row PARITY
(core h takes global rows s with s % 2 == h) so causal work is exactly
balanced and the SPMD program is uniform (per-core differences are pure data).

Speed (~75.2us cost-model vs 211.7us bf16 baseline):
  - all heavy matmuls run in fp8e4m3 with MatmulPerfMode.DoubleRow
    (2 contraction rows packed per partition, 0.5 PE cycles per output
    row — 4x bf16 MAC throughput); fp32 PSUM accumulation throughout
  - PSUM is evacuated in [P,2,512] double-bank units so the DVE/ACT
    conversion copies pay per-op access latency half as often
  - a few dummy matmuls on zeroed constants during the initial DMA wait
    walk the PE through its p-state ramp so real work starts at 2.4GHz
  - AV for the first block of each pair is emitted mid-score-loop so its
    stores drain early; denominator chains + reciprocal run before the
    AV pos chains so the final scale starts right at the last matmul

Accuracy (rel err vs fp32 reference ~7.6e-3, gate 2e-2):
  - weights are host-prescaled by WS=32 (exact power of 2) so that the fp8
    RESIDUAL split W*32 = whi + wlo stays out of e4m3 denormal underflow;
    the scale is folded back via the exp scale (1/WS^2 for scores) and the
    denominator (ones vector = WS).
  - a bf16 "side path" covers the earliest rows, where softmax averaging
    cannot damp fp8's ~5% per-element error: K/V for keys 0:SW and Q for
    local queries 0:SW (SW=64 -> global rows 0:128) are computed via 3
    fp8-residual chains (hi@hi + hi@lo + lo@hi), stored bf16;
    block0/qc0's chunk-0 x [0:SW,0:SW] scores, exp, AV and denominator
    run in bf16 off those tiles, and that region of the fp8 P tile is
    zeroed so the DoubleRow AV pairs skip it. Side tiles are zero-padded
    to 128 partitions so every matmul keeps full-partition shapes.
  - softmax needs no max-subtraction: logits are bounded (~|2.5| pre-scale)
    and P=exp() lands in [~0.16, ~7], comfortably inside e4m3 range.

Layout (PE matmul: out = lhsT.T @ rhs, contraction on the 128-partition dim):
  host passes x^T and W^T so the contraction dim d lands on partitions with
  zero on-device transposes. Kt[e,k], Qt[e,q] come out of the projections
  with e on partitions; scores are computed transposed St[k,q]; the causal
  mask is one additive -1e30 window tile [128,64] reused by every diagonal
  chunk (keep iff p <= 2j+h is m-independent); the softmax denominator is a
  ones-vector matmul (PE reduces over the key partition dim in fp32);
  the final 1/denom scale rides the PSUM->SBUF copy; out is stored bf16.
"""

import numpy as np

B, S, D, P = 4, 2048, 1024, 128
SW = 64            # side-path width (keys/local queries covered in bf16)
NQ = S // 2          # queries per core (parity split)
QT = 256             # score-tile width in (core-local) query dim
NEG = -1e30
WS = 32.0            # host weight prescale (exact power of 2)
N_CORES = 8

_cache = {}


def _build():
    import concourse.mybir as mybir
    import concourse.tile as tile
    from concourse import bacc

    f32 = mybir.dt.float32
    bf = mybir.dt.bfloat16
    f8 = mybir.dt.float8e4
    DR = mybir.MatmulPerfMode.DoubleRow

    nc = bacc.Bacc()

    x8 = nc.dram_tensor("x8", [D, S], f8, kind="ExternalInput")
    xq8 = nc.dram_tensor("xq8", [D, NQ], f8, kind="ExternalInput")
    wq8 = nc.dram_tensor("wq8", [D, D], f8, kind="ExternalInput")
    wk8 = nc.dram_tensor("wk8", [D, D], f8, kind="ExternalInput")
    wv8 = nc.dram_tensor("wv8", [D, D], f8, kind="ExternalInput")
    wql = nc.dram_tensor("wql", [D, D], f8, kind="ExternalInput")
    wkl = nc.dram_tensor("wkl", [D, D], f8, kind="ExternalInput")
    wvl = nc.dram_tensor("wvl", [D, D], f8, kind="ExternalInput")
    xs8 = nc.dram_tensor("xs8", [D, SW], f8, kind="ExternalInput")
    xsl = nc.dram_tensor("xsl", [D, SW], f8, kind="ExternalInput")
    xqs8 = nc.dram_tensor("xqs8", [D, SW], f8, kind="ExternalInput")
    xqsl = nc.dram_tensor("xqsl", [D, SW], f8, kind="ExternalInput")
    maskw = nc.dram_tensor("maskw", [P, 64], f32, kind="ExternalInput")
    out = nc.dram_tensor("out", [NQ, D], bf, kind="ExternalOutput")

    x3 = x8.ap().rearrange("(do di) s -> di do s", di=P)
    xq3 = xq8.ap().rearrange("(do di) s -> di do s", di=P)
    wq3 = wq8.ap().rearrange("(do di) e -> di do e", di=P)
    wk3 = wk8.ap().rearrange("(do di) e -> di do e", di=P)
    wv3 = wv8.ap().rearrange("(do di) e -> di do e", di=P)
    wql3 = wql.ap().rearrange("(do di) e -> di do e", di=P)
    wkl3 = wkl.ap().rearrange("(do di) e -> di do e", di=P)
    wvl3 = wvl.ap().rearrange("(do di) e -> di do e", di=P)
    xs3 = xs8.ap().rearrange("(do di) s -> di do s", di=P)
    xsl3 = xsl.ap().rearrange("(do di) s -> di do s", di=P)
    xqs3 = xqs8.ap().rearrange("(do di) s -> di do s", di=P)
    xqsl3 = xqsl.ap().rearrange("(do di) s -> di do s", di=P)
    out3 = out.ap().rearrange("q (eh ei) -> q eh ei", eh=2)

    EXP = mybir.ActivationFunctionType.Exp
    SC8 = float(1.0 / np.sqrt(np.float32(D)) / (WS * WS))

    with tile.TileContext(nc) as tc:
        with (
            tc.tile_pool(name="const", bufs=1) as const_pool,
            tc.tile_pool(name="w", bufs=1) as w_pool,
            tc.tile_pool(name="ins", bufs=1) as ins_pool,
            tc.tile_pool(name="prod", bufs=1) as prod,
        ):
            # ---- all input DMAs up front, in first-use priority order.
            # x slab 0 and a thin first wk slice (e cols 0:256) lead so the
            # first projection pair-tile can start ~3us in.
            xsb = [
                ins_pool.tile([P, 8, 512], f8, name=f"x{s}") for s in range(4)
            ]
            wk_sb = w_pool.tile([P, 8, D], f8, name="wk_sb")
            # do-half interleaved first loads: the di-outer matmul order on
            # slab 0 starts on (x do 0:4, wk e 0:256 do 0:4) alone
            nc.sync.dma_start(xsb[0][:, 0:4, :], x3[:, 0:4, 0:512])
            nc.sync.dma_start(wk_sb[:, 0:4, 0:256], wk3[:, 0:4, 0:256])
            nc.sync.dma_start(xsb[0][:, 4:8, :], x3[:, 4:8, 0:512])
            nc.sync.dma_start(wk_sb[:, 4:8, 0:256], wk3[:, 4:8, 0:256])
            nc.sync.dma_start(wk_sb[:, 0:4, 256:D], wk3[:, 0:4, 256:D])
            nc.sync.dma_start(wk_sb[:, 4:8, 256:D], wk3[:, 4:8, 256:D])
            for s in range(1, 4):
                nc.sync.dma_start(xsb[s][:], x3[:, :, s * 512 : (s + 1) * 512])
            wv_sb = w_pool.tile([P, 8, D], f8, name="wv_sb")
            nc.sync.dma_start(wv_sb[:, 0:4, :], wv3[:, 0:4, :])
            nc.sync.dma_start(wv_sb[:, 4:8, :], wv3[:, 4:8, :])
            xq_sb = ins_pool.tile([P, 8, NQ], f8, name="xq_sb")
            nc.sync.dma_start(xq_sb[:], xq3)
            wq_sb = w_pool.tile([P, 8, D], f8, name="wq_sb")
            nc.sync.dma_start(wq_sb[:, 0:4, :], wq3[:, 0:4, :])
            nc.sync.dma_start(wq_sb[:, 4:8, :], wq3[:, 4:8, :])
            xs_sb = ins_pool.tile([P, 8, SW], f8, name="xs_sb")
            nc.sync.dma_start(xs_sb[:], xs3)
            xsl_sb = ins_pool.tile([P, 8, SW], f8, name="xsl_sb")
            nc.sync.dma_start(xsl_sb[:], xsl3)
            xqs_sb = ins_pool.tile([P, 8, SW], f8, name="xqs_sb")
            nc.sync.dma_start(xqs_sb[:], xqs3)
            xqsl_sb = ins_pool.tile([P, 8, SW], f8, name="xqsl_sb")
            nc.sync.dma_start(xqsl_sb[:], xqsl3)
            wkl_sb = w_pool.tile([P, 8, D], f8, name="wkl_sb")
            nc.sync.dma_start(wkl_sb[:], wkl3)
            wql_sb = w_pool.tile([P, 8, D], f8, name="wql_sb")
            nc.sync.dma_start(wql_sb[:], wql3)
            wvl_sb = w_pool.tile([P, 8, D], f8, name="wvl_sb")
            nc.sync.dma_start(wvl_sb[:], wvl3)
            mask_sb = const_pool.tile([P, 64], f32)
            nc.sync.dma_start(mask_sb[:], maskw.ap())

            # constants + exp-table warm while DMAs land
            warm = const_pool.tile([P, 1], f32)
            nc.vector.memset(warm[:], 0.0)
            nc.scalar.activation(out=warm[:], in_=warm[:], func=EXP, scale=1.0)
            ones8 = const_pool.tile([P, 2, 1], f8)
            nc.vector.memset(ones8[:], WS)
            # p-state pre-warm: ~3us of dummy matmuls on memset constants
            # while the first DMAs land, so the real projections start at
            # the PE's full 2.4GHz instead of ramping through them
            wrm = const_pool.tile([P, 512], bf)
            nc.vector.memset(wrm[:], 0.0)
            onesb = const_pool.tile([P, 1], bf)
            nc.vector.memset(onesb[:], WS)

            # persistent products
            kt_slabs = [
                prod.tile([P, 8, 512], f8, tag=f"kt{s}", name=f"kt{s}")
                for s in range(4)
            ]
            v_slabs = [
                prod.tile([P, 4, 2, 512], f8, tag=f"v{s}", name=f"v{s}")
                for s in range(4)
            ]
            qt8_sb = prod.tile([P, 8, 2, 512], f8, tag="qt")
            ktb = prod.tile([P, 8, SW], bf, tag="ktb")
            qtb = prod.tile([P, 8, SW], bf, tag="qtb")
            vb = prod.tile([P, 2, 512], bf, tag="vb")
            ptb = prod.tile([P, P], bf, tag="ptb")
            # zero-pad the side tiles so all matmuls stay 128-partition:
            # ptb rows/cols >= SW and vb rows >= SW contribute exact zeros
            nc.gpsimd.memset(ptb[:], 0.0)
            nc.gpsimd.memset(vb[SW:P, :, :], 0.0)

            # Pool/GPSIMD cannot access PSUM -> PSUM-source copies go on
            # DVE/ACT only (walrus birverifier enforces this).
            copy_engines = [nc.vector, nc.scalar]
            copy_i = [0]

            def copy_out(dst, src):
                eng = copy_engines[copy_i[0] % 2]
                copy_i[0] += 1
                if eng is nc.scalar:
                    eng.copy(out=dst, in_=src)
                else:
                    eng.tensor_copy(out=dst, in_=src)

            with (
                tc.tile_pool(name="pt", bufs=4) as pt_pool,
                tc.tile_pool(name="ob", bufs=4) as ob_pool,
                tc.tile_pool(name="rc", bufs=4) as rc_pool,
            ):
                # phase-1 PSUM pools; released before attention opens its own
                pj = tc.alloc_tile_pool(name="pj", bufs=3, space="PSUM")
                pjs = tc.alloc_tile_pool(name="pjs", bufs=2, space="PSUM")
                wps = pj.tile([P, 2, 512], f32, tag="pj", name="wps")
                for i in range(5):
                    nc.tensor.matmul(
                        wps[0:2, 0, :], wrm[:, 0:2], wrm[:],
                        start=(i == 0), stop=(i == 4), perf_mode=None,
                    )
                # ---- K^T projection: kt[e on partitions, k] ----
                for s in range(4):
                    for ep in range(4):  # ec pair = (2ep, 2ep+1)
                        ps = pj.tile([P, 2, 512], f32, tag="pj", name="pj")
                        # di-outer on slab 0 so the first matmuls need only
                        # the do 0:4 halves of x/wk (they land first)
                        order = (
                            [(di, half) for di in range(4) for half in range(2)]
                            if s == 0
                            else [(di, half) for half in range(2) for di in range(4)]
                        )
                        for di, half in order:
                            ec = 2 * ep + half
                            do = 2 * di
                            nc.tensor.matmul(
                                ps[:, half, :],
                                wk_sb[:, do : do + 2, ec * P : (ec + 1) * P],
                                xsb[s][:, do : do + 2, :],
                                start=(di == 0),
                                stop=(di == 3),
                                perf_mode=DR,
                            )
                        copy_out(kt_slabs[s][:, 2 * ep : 2 * ep + 2, :], ps[:])

                # ---- V projection: v[k on partitions, e] ----
                for s in range(4):
                    for kq in range(4):
                        ps = pj.tile([P, 2, 512], f32, tag="pj", name="pj")
                        for es in range(2):
                            for di in range(4):
                                do = 2 * di
                                nc.tensor.matmul(
                                    ps[:, es, :],
                                    xsb[s][:, do : do + 2, kq * P : (kq + 1) * P],
                                    wv_sb[:, do : do + 2, es * 512 : (es + 1) * 512],
                                    start=(di == 0),
                                    stop=(di == 3),
                                    perf_mode=DR,
                                )
                        copy_out(v_slabs[s][:, kq, :, :], ps[:])

                # ---- Q^T projection: qt[e on partitions, q] ----
                for ec in range(8):
                    ps = pj.tile([P, 2, 512], f32, tag="pj", name="pj")
                    for qs in range(2):
                        for di in range(4):
                            do = 2 * di
                            nc.tensor.matmul(
                                ps[:, qs, :],
                                wq_sb[:, do : do + 2, ec * P : (ec + 1) * P],
                                xq_sb[:, do : do + 2, qs * 512 : (qs + 1) * 512],
                                start=(di == 0),
                                stop=(di == 3),
                                perf_mode=DR,
                            )
                    copy_out(qt8_sb[:, ec, :, :], ps[:])

                # ---- side path: fp8-residual chains -> bf16 tiles ----
                for tgt, whi_sb, wlo_sb, ahi_sb, alo_sb in (
                    (ktb, wk_sb, wkl_sb, xs_sb, xsl_sb),
                    (qtb, wq_sb, wql_sb, xqs_sb, xqsl_sb),
                ):
                    for ep in range(4):
                        psb = pjs.tile([P, 2, SW], f32, tag="pjs", name="psb")
                        for half in range(2):
                            ec = 2 * ep + half
                            chains = (
                                (whi_sb, ahi_sb), (wlo_sb, ahi_sb),
                                (whi_sb, alo_sb),
                            )
                            for ci, (wt, at) in enumerate(chains):
                                for di in range(4):
                                    do = 2 * di
                                    nc.tensor.matmul(
                                        psb[:, half, :],
                                        wt[:, do : do + 2, ec * P : (ec + 1) * P],
                                        at[:, do : do + 2, :],
                                        start=(ci == 0 and di == 0),
                                        stop=(ci == 2 and di == 3),
                                        perf_mode=DR,
                                    )
                        copy_out(tgt[:, 2 * ep : 2 * ep + 2, :], psb[:])


                ps = pj.tile([P, 2, 512], f32, tag="pj", name="pj")
                for es in range(2):  # vb[k 0:SW, e]
                    chains = ((xs_sb, wv_sb), (xsl_sb, wv_sb), (xs_sb, wvl_sb))
                    for ci, (at, wt) in enumerate(chains):
                        for di in range(4):
                            do = 2 * di
                            nc.tensor.matmul(
                                ps[0:SW, es, :],
                                at[:, do : do + 2, :],
                                wt[:, do : do + 2, es * 512 : (es + 1) * 512],
                                start=(ci == 0 and di == 0),
                                stop=(ci == 2 and di == 3),
                                perf_mode=DR,
                            )
                copy_out(vb[0:SW, :, :], ps[0:SW, :, :])

                # ---- attention: swap PSUM pool layout ----
                pjs.release()
                pj.release()
                ps_pool = tc.alloc_tile_pool(name="ps", bufs=3, space="PSUM")
                po_pool = tc.alloc_tile_pool(name="po", bufs=2, space="PSUM")
                pd_pool = tc.alloc_tile_pool(name="pd", bufs=1, space="PSUM")

                # side scores: chunk0 keys x qc0 cols of block 0, bf16
                sps = ps_pool.tile([P, QT], f32, tag="ps", name="sps")
                for ec in range(8):
                    nc.tensor.matmul(
                        sps[0:SW, 0:SW],
                        ktb[:, ec, :],
                        qtb[:, ec, :],
                        start=(ec == 0),
                        stop=(ec == 7),
                    )
                nc.vector.tensor_add(
                    out=sps[0:SW, 0:SW], in0=sps[0:SW, 0:SW],
                    in1=mask_sb[0:SW, 0:SW],
                )
                nc.scalar.activation(
                    out=ptb[0:SW, 0:SW], in_=sps[0:SW, 0:SW],
                    func=EXP, scale=SC8,
                )

                def emit_av(q, pt, last_tile=False):
                    for qc in range(2):
                        nk = 4 * q + 2 if qc == 0 else 4 * q + 4
                        npair = nk // 2
                        blk0 = q == 0 and qc == 0
                        dn = pd_pool.tile([P, 1], f32, tag="dn")
                        pos = po_pool.tile(
                            [P, 2, 512], f32, tag="po", name="po"
                        )
                        # denominator chain + reciprocal first: rc is ready
                        # long before the pos chains finish, so the scale
                        # starts right at the last AV matmul
                        if blk0:
                            nc.tensor.matmul(
                                dn[:], ptb[:], onesb[:],
                                start=True, stop=False,
                            )
                        for pr in range(npair):
                            nc.tensor.matmul(
                                dn[:],
                                pt[:, 2 * pr : 2 * pr + 2, qc * P : (qc + 1) * P],
                                ones8[:],
                                start=(pr == 0 and not blk0),
                                stop=(pr == npair - 1),
                                perf_mode=DR,
                            )
                        rc = rc_pool.tile([P, 1], f32, tag="rc")
                        nc.vector.reciprocal(out=rc[:], in_=dn[:])
                        if blk0:
                            for es in range(2):
                                nc.tensor.matmul(
                                    pos[:, es, :],
                                    ptb[:],
                                    vb[:, es, :],
                                    start=True,
                                    stop=False,
                                )
                        for pr in range(npair):
                            first = pr == 0 and not blk0
                            last = pr == npair - 1
                            s2, kqp = pr // 2, (pr % 2) * 2
                            lhsT = pt[:, 2 * pr : 2 * pr + 2, qc * P : (qc + 1) * P]
                            for es in range(2):
                                nc.tensor.matmul(
                                    pos[:, es, :],
                                    lhsT,
                                    v_slabs[s2][:, kqp : kqp + 2, es, :],
                                    start=first,
                                    stop=last,
                                    perf_mode=DR,
                                )
                        q0 = q * QT + qc * P
                        if last_tile and qc == 1:
                            # split the final scale across DVE/ACT so the
                            # tail drains in parallel
                            for es in range(2):
                                ob = ob_pool.tile([P, 2, 512], bf, tag="ob", name="ob")
                                if es == 0:
                                    nc.vector.tensor_scalar_mul(
                                        out=ob[:, 0, :], in0=pos[:, 0, :],
                                        scalar1=rc[:],
                                    )
                                else:
                                    nc.scalar.activation(
                                        out=ob[:, 1, :], in_=pos[:, 1, :],
                                        func=mybir.ActivationFunctionType.Copy,
                                        scale=rc[:],
                                    )
                                nc.sync.dma_start(
                                    out3[q0 : q0 + P, es, :], ob[:, es, :]
                                )
                        else:
                            ob = ob_pool.tile([P, 2, 512], bf, tag="ob", name="ob")
                            nc.vector.tensor_scalar_mul(
                                out=ob[:], in0=pos[:], scalar1=rc[:]
                            )
                            nc.sync.dma_start(out3[q0 : q0 + P, :, :], ob[:])

                for a, b in ((0, 1), (2, 3)):
                    pts = {
                        q: pt_pool.tile([P, 16, QT], f8, tag="pt", name=f"pt{q}")
                        for q in (a, b)
                    }
                    def score_mms(q, kc, dst, c0):
                        s, kq = kc // 4, kc % 4
                        qh, qo = q // 2, (q % 2) * QT
                        for di in range(4):
                            ec = 2 * di
                            nc.tensor.matmul(
                                dst,
                                kt_slabs[s][:, ec : ec + 2, kq * P : (kq + 1) * P],
                                qt8_sb[:, ec : ec + 2, qh, qo + c0 : qo + QT],
                                start=(di == 0),
                                stop=(di == 3),
                                perf_mode=DR,
                            )

                    for kcp in range(2 * b + 2):
                        kc0, kc1 = 2 * kcp, 2 * kcp + 1
                        active = [q for q in (a, b) if kc0 < 4 * q + 4]
                        # non-diag kc pairs share one single-bank [P,2,QT]
                        # psum and a single exp, halving the per-op ACT
                        # overhead on the phase-2 critical path
                        work = []  # (q, psum, kind)
                        for q in active:
                            if kc1 < 4 * q:  # both tiles non-diag
                                ps2 = ps_pool.tile(
                                    [P, 2, QT], f32, tag="ps", name="ps"
                                )
                                score_mms(q, kc0, ps2[:, 0, :], 0)
                                score_mms(q, kc1, ps2[:, 1, :], 0)
                                work.append((q, ps2, "pair"))
                            else:
                                for kc in (kc0, kc1):
                                    if kc >= 4 * q + 4:
                                        continue
                                    m = kc - 4 * q
                                    c0 = 64 * m if m >= 0 else 0
                                    ps1 = ps_pool.tile(
                                        [P, 2, QT], f32, tag="ps", name="ps"
                                    )
                                    score_mms(q, kc, ps1[:, 0, c0:QT], c0)
                                    work.append((q, ps1, kc))
                        for q, psx, kind in work:
                            if kind == "pair":
                                nc.scalar.activation(
                                    out=pts[q][:, 2 * kcp : 2 * kcp + 2, :],
                                    in_=psx[:],
                                    func=EXP,
                                    scale=SC8,
                                )
                                continue
                            kc = kind
                            m = kc - 4 * q
                            c0 = 64 * m if m >= 0 else 0
                            if m >= 0:
                                w0 = 64 * m
                                nc.vector.tensor_add(
                                    out=psx[:, 0, w0 : w0 + 64],
                                    in0=psx[:, 0, w0 : w0 + 64],
                                    in1=mask_sb[:],
                                )
                            nc.scalar.activation(
                                out=pts[q][:, kc, c0:QT],
                                in_=psx[:, 0, c0:QT],
                                func=EXP,
                                scale=SC8,
                            )
                            if q == 0 and kc == 0:
                                # side path provides keys 0:SW x cols 0:SW
                                nc.gpsimd.memset(pts[q][0:SW, 0, 0:SW], 0.0)
                            if m == 1:
                                nc.gpsimd.memset(pts[q][:, kc, 0:64], 0.0)
                            elif m == 3:
                                nc.gpsimd.memset(pts[q][:, kc, 128:192], 0.0)
                        if kc1 == 4 * a + 3:
                            # block a's chunks are complete: emit its AV now
                            # so the post-processing/stores drain early
                            emit_av(a, pts[a])
                    emit_av(b, pts[b], last_tile=(b == 3))
                pd_pool.release()
                po_pool.release()
                ps_pool.release()

    nc.compile()
    return nc


def _get_nc():
    if "nc" not in _cache:
        _cache["nc"] = _build()
    return _cache["nc"]


def _host_mask_window(h: int) -> np.ndarray:
    # one additive window tile [p, jj], reused by every diagonal chunk m of
    # every query block i: for score tile at key chunk kc = 4i+m, query
    # block i, the masked/boundary region is cols j in [64m, 64m+64) and
    # keep(p, j) iff p - 2j <= h - 128m  <=>  p <= 2*jj + h  (m cancels).
    p = np.arange(P)[:, None]
    jj = np.arange(64)[None, :]
    keep = p <= (2 * jj + h)
    return np.where(keep, np.float32(0.0), np.float32(NEG)).astype(np.float32)


def make_in_maps(x, Wq, Wk, Wv):
    import ml_dtypes

    F8 = ml_dtypes.float8_e4m3

    def split8(a):
        # a fp32 -> hi/lo fp8 residual pair (same orientation)
        a = np.ascontiguousarray(np.asarray(a, dtype=np.float32))
        hi = a.astype(F8)
        lo = (a - hi.astype(np.float32)).astype(F8)
        return hi, lo

    wqhi, wqlo = split8(np.asarray(Wq, dtype=np.float32).T * WS)
    wkhi, wklo = split8(np.asarray(Wk, dtype=np.float32).T * WS)
    wvhi, wvlo = split8(np.asarray(Wv, dtype=np.float32).T * WS)
    masks_h = [_host_mask_window(0), _host_mask_window(1)]
    in_maps = []
    for c in range(N_CORES):
        b, h = c // 2, c % 2
        xb = np.asarray(x[b], dtype=np.float32)
        xbT = np.ascontiguousarray(xb.T)
        xqT = np.ascontiguousarray(xb[h::2, :].T)
        xshi, xslo = split8(xb[0:SW, :].T)
        xqshi, xqslo = split8(xb[h::2, :][0:SW, :].T)
        in_maps.append(
            {
                "x8": xbT.astype(F8),
                "xq8": xqT.astype(F8),
                "wq8": wqhi, "wql": wqlo,
                "wk8": wkhi, "wkl": wklo,
                "wv8": wvhi, "wvl": wvlo,
                "xs8": xshi, "xsl": xslo,
                "xqs8": xqshi, "xqsl": xqslo,
                "maskw": masks_h[h],
            }
        )
    return in_maps


def kernel(x, Wq, Wk, Wv):
    from concourse.bass_utils import run_bass_kernel_spmd

    nc = _get_nc()
    in_maps = make_in_maps(x, Wq, Wk, Wv)
    res = run_bass_kernel_spmd(nc, in_maps, core_ids=list(range(N_CORES)))
    out = np.empty((B, S, D), dtype=np.float32)
    for c in range(N_CORES):
        b, h = c // 2, c % 2
        out[b, h::2, :] = res.results[c]["out"].astype(np.float32)
    return out


# revision 64
# speedup vs baseline: 1.0002x; 1.0002x over previous
"""Causal attention kernel for 8 Trainium2 NeuronCores — fp8 DoubleRow edition.

Problem: x[4,2048,1024] fp32, Wq/Wk/Wv[1024,1024] fp32 (nn.Linear: y = x @ W.T),
single-head causal attention, softmax(QK^T/sqrt(D)) @ V.

Sharding: 2 cores per batch; within a batch, queries are split by row PARITY
(core h takes global rows s with s % 2 == h) so causal work is exactly
balanced and the SPMD program is uniform (per-core differences are pure data).

Speed (~72.8us cost-model vs 211.7us bf16 baseline):
  - all heavy matmuls run in fp8e4m3 with MatmulPerfMode.DoubleRow
    (2 contraction rows packed per partition, 0.5 PE cycles per output
    row — 4x bf16 MAC throughput); fp32 PSUM accumulation throughout
  - PSUM is evacuated in [P,2,512] double-bank units so the DVE/ACT
    conversion copies pay per-op access latency half as often
  - a few dummy matmuls on zeroed constants during the initial DMA wait
    walk the PE through its p-state ramp so real work starts at 2.4GHz
  - AV for the first block of each pair is emitted mid-score-loop so its
    stores drain early; denominator chains + reciprocal run before the
    AV pos chains so the final scale starts right at the last matmul
  - non-diagonal score-tile PAIRS share one single-bank [P,2,256] psum
    and a single exp op, halving the per-op ACT access-latency overhead
    that otherwise left the scalar engine with zero slack in phase 2

Accuracy (rel err vs fp32 reference ~7.6e-3, gate 2e-2):
  - weights are host-prescaled by WS=32 (exact power of 2) so that the fp8
    RESIDUAL split W*32 = whi + wlo stays out of e4m3 denormal underflow;
    the scale is folded back via the exp scale (1/WS^2 for scores) and the
    denominator (ones vector = WS).
  - a bf16 "side path" covers the earliest rows, where softmax averaging
    cannot damp fp8's ~5% per-element error: K/V for keys 0:SW and Q for
    local queries 0:SW (SW=64 -> global rows 0:128) are computed via 3
    fp8-residual chains (hi@hi + hi@lo + lo@hi), stored bf16;
    block0/qc0's chunk-0 x [0:SW,0:SW] scores, exp, AV and denominator
    run in bf16 off those tiles, and that region of the fp8 P tile is
    zeroed so the DoubleRow AV pairs skip it. Side tiles are zero-padded
    to 128 partitions so every matmul keeps full-partition shapes.
  - softmax needs no max-subtraction: logits are bounded (~|2.5| pre-scale)
    and P=exp() lands in [~0.16, ~7], comfortably inside e4m3 range.

Layout (PE matmul: out = lhsT.T @ rhs, contraction on the 128-partition dim):
  host passes x^T and W^T so the contraction dim d lands on partitions with
  zero on-device transposes. Kt[e,k], Qt[e,q] come out of the projections
  with e on partitions; scores are computed transposed St[k,q]; the causal
  mask is one additive -1e30 window tile [128,64] reused by every diagonal
  chunk (keep iff p <= 2j+h is m-independent); the softmax denominator is a
  ones-vector matmul (PE reduces over the key partition dim in fp32);
  the final 1/denom scale rides the PSUM->SBUF copy; out is stored bf16.
"""

import numpy as np

B, S, D, P = 4, 2048, 1024, 128
SW = 64            # side-path width (keys/local queries covered in bf16)
NQ = S // 2          # queries per core (parity split)
QT = 256             # score-tile width in (core-local) query dim
NEG = -1e30
WS = 32.0            # host weight prescale (exact power of 2)
N_CORES = 8

_cache = {}


def _build():
    import concourse.mybir as mybir
    import concourse.tile as tile
    from concourse import bacc

    f32 = mybir.dt.float32
    bf = mybir.dt.bfloat16
    f8 = mybir.dt.float8e4
    DR = mybir.MatmulPerfMode.DoubleRow

    nc = bacc.Bacc()

    x8 = nc.dram_tensor("x8", [D, S], f8, kind="ExternalInput")
    xq8 = nc.dram_tensor("xq8", [D, NQ], f8, kind="ExternalInput")
    wq8 = nc.dram_tensor("wq8", [D, D], f8, kind="ExternalInput")
    wk8 = nc.dram_tensor("wk8", [D, D], f8, kind="ExternalInput")
    wv8 = nc.dram_tensor("wv8", [D, D], f8, kind="ExternalInput")
    wql = nc.dram_tensor("wql", [D, D], f8, kind="ExternalInput")
    wkl = nc.dram_tensor("wkl", [D, D], f8, kind="ExternalInput")
    wvl = nc.dram_tensor("wvl", [D, D], f8, kind="ExternalInput")
    xs8 = nc.dram_tensor("xs8", [D, SW], f8, kind="ExternalInput")
    xsl = nc.dram_tensor("xsl", [D, SW], f8, kind="ExternalInput")
    xqs8 = nc.dram_tensor("xqs8", [D, SW], f8, kind="ExternalInput")
    xqsl = nc.dram_tensor("xqsl", [D, SW], f8, kind="ExternalInput")
    maskw = nc.dram_tensor("maskw", [P, 64], f32, kind="ExternalInput")
    out = nc.dram_tensor("out", [NQ, D], bf, kind="ExternalOutput")

    x3 = x8.ap().rearrange("(do di) s -> di do s", di=P)
    xq3 = xq8.ap().rearrange("(do di) s -> di do s", di=P)
    wq3 = wq8.ap().rearrange("(do di) e -> di do e", di=P)
    wk3 = wk8.ap().rearrange("(do di) e -> di do e", di=P)
    wv3 = wv8.ap().rearrange("(do di) e -> di do e", di=P)
    wql3 = wql.ap().rearrange("(do di) e -> di do e", di=P)
    wkl3 = wkl.ap().rearrange("(do di) e -> di do e", di=P)
    wvl3 = wvl.ap().rearrange("(do di) e -> di do e", di=P)
    xs3 = xs8.ap().rearrange("(do di) s -> di do s", di=P)
    xsl3 = xsl.ap().rearrange("(do di) s -> di do s", di=P)
    xqs3 = xqs8.ap().rearrange("(do di) s -> di do s", di=P)
    xqsl3 = xqsl.ap().rearrange("(do di) s -> di do s", di=P)
    out3 = out.ap().rearrange("q (eh ei) -> q eh ei", eh=2)

    EXP = mybir.ActivationFunctionType.Exp
    SC8 = float(1.0 / np.sqrt(np.float32(D)) / (WS * WS))

    with tile.TileContext(nc) as tc:
        with (
            tc.tile_pool(name="const", bufs=1) as const_pool,
            tc.tile_pool(name="w", bufs=1) as w_pool,
            tc.tile_pool(name="ins", bufs=1) as ins_pool,
            tc.tile_pool(name="prod", bufs=1) as prod,
        ):
            # ---- all input DMAs up front, in first-use priority order.
            # x slab 0 and a thin first wk slice (e cols 0:256) lead so the
            # first projection pair-tile can start ~3us in.
            xsb = [
                ins_pool.tile([P, 8, 512], f8, name=f"x{s}") for s in range(4)
            ]
            wk_sb = w_pool.tile([P, 8, D], f8, name="wk_sb")
            # do-half interleaved first loads: the di-outer matmul order on
            # slab 0 starts on (x do 0:4, wk e 0:256 do 0:4) alone
            nc.sync.dma_start(xsb[0][:, 0:4, :], x3[:, 0:4, 0:512])
            nc.sync.dma_start(wk_sb[:, 0:4, 0:256], wk3[:, 0:4, 0:256])
            nc.sync.dma_start(xsb[0][:, 4:8, :], x3[:, 4:8, 0:512])
            nc.sync.dma_start(wk_sb[:, 4:8, 0:256], wk3[:, 4:8, 0:256])
            nc.sync.dma_start(wk_sb[:, 0:4, 256:D], wk3[:, 0:4, 256:D])
            nc.sync.dma_start(wk_sb[:, 4:8, 256:D], wk3[:, 4:8, 256:D])
            for s in range(1, 4):
                nc.sync.dma_start(xsb[s][:], x3[:, :, s * 512 : (s + 1) * 512])
            wv_sb = w_pool.tile([P, 8, D], f8, name="wv_sb")
            nc.sync.dma_start(wv_sb[:, 0:4, :], wv3[:, 0:4, :])
            nc.sync.dma_start(wv_sb[:, 4:8, :], wv3[:, 4:8, :])
            xq_sb = ins_pool.tile([P, 8, NQ], f8, name="xq_sb")
            nc.sync.dma_start(xq_sb[:], xq3)
            wq_sb = w_pool.tile([P, 8, D], f8, name="wq_sb")
            nc.sync.dma_start(wq_sb[:, 0:4, :], wq3[:, 0:4, :])
            nc.sync.dma_start(wq_sb[:, 4:8, :], wq3[:, 4:8, :])
            xs_sb = ins_pool.tile([P, 8, SW], f8, name="xs_sb")
            nc.sync.dma_start(xs_sb[:], xs3)
            xsl_sb = ins_pool.tile([P, 8, SW], f8, name="xsl_sb")
            nc.sync.dma_start(xsl_sb[:], xsl3)
            xqs_sb = ins_pool.tile([P, 8, SW], f8, name="xqs_sb")
            nc.sync.dma_start(xqs_sb[:], xqs3)
            xqsl_sb = ins_pool.tile([P, 8, SW], f8, name="xqsl_sb")
            nc.sync.dma_start(xqsl_sb[:], xqsl3)
            wkl_sb = w_pool.tile([P, 8, D], f8, name="wkl_sb")
            nc.sync.dma_start(wkl_sb[:], wkl3)
            wql_sb = w_pool.tile([P, 8, D], f8, name="wql_sb")
            nc.sync.dma_start(wql_sb[:], wql3)
            wvl_sb = w_pool.tile([P, 8, D], f8, name="wvl_sb")
            nc.sync.dma_start(wvl_sb[:], wvl3)
            mask_sb = const_pool.tile([P, 64], f32)
            nc.sync.dma_start(mask_sb[:], maskw.ap())

            # constants + exp-table warm while DMAs land
            warm = const_pool.tile([P, 1], f32)
            nc.vector.memset(warm[:], 0.0)
            nc.scalar.activation(out=warm[:], in_=warm[:], func=EXP, scale=1.0)
            ones8 = const_pool.tile([P, 2, 1], f8)
            nc.vector.memset(ones8[:], WS)
            # p-state pre-warm: ~3us of dummy matmuls on memset constants
            # while the first DMAs land, so the real projections start at
            # the PE's full 2.4GHz instead of ramping through them
            wrm = const_pool.tile([P, 512], bf)
            nc.vector.memset(wrm[:], 0.0)
            onesb = const_pool.tile([P, 1], bf)
            nc.vector.memset(onesb[:], WS)

            # persistent products
            kt_slabs = [
                prod.tile([P, 8, 512], f8, tag=f"kt{s}", name=f"kt{s}")
                for s in range(4)
            ]
            v_slabs = [
                prod.tile([P, 4, 2, 512], f8, tag=f"v{s}", name=f"v{s}")
                for s in range(4)
            ]
            qt8_sb = prod.tile([P, 8, 2, 512], f8, tag="qt")
            ktb = prod.tile([P, 8, SW], bf, tag="ktb")
            qtb = prod.tile([P, 8, SW], bf, tag="qtb")
            vb = prod.tile([P, 2, 512], bf, tag="vb")
            ptb = prod.tile([P, P], bf, tag="ptb")
            # zero-pad the side tiles so all matmuls stay 128-partition:
            # ptb rows/cols >= SW and vb rows >= SW contribute exact zeros
            nc.gpsimd.memset(ptb[:], 0.0)
            nc.gpsimd.memset(vb[SW:P, :, :], 0.0)

            # Pool/GPSIMD cannot access PSUM -> PSUM-source copies go on
            # DVE/ACT only (walrus birverifier enforces this).
            copy_engines = [nc.vector, nc.scalar]
            copy_i = [0]

            def copy_out(dst, src):
                eng = copy_engines[copy_i[0] % 2]
                copy_i[0] += 1
                if eng is nc.scalar:
                    eng.copy(out=dst, in_=src)
                else:
                    eng.tensor_copy(out=dst, in_=src)

            with (
                tc.tile_pool(name="pt", bufs=4) as pt_pool,
                tc.tile_pool(name="ob", bufs=4) as ob_pool,
                tc.tile_pool(name="rc", bufs=4) as rc_pool,
            ):
                # phase-1 PSUM pools; released before attention opens its own
                pj = tc.alloc_tile_pool(name="pj", bufs=3, space="PSUM")
                pjs = tc.alloc_tile_pool(name="pjs", bufs=2, space="PSUM")
                wps = pj.tile([P, 2, 512], f32, tag="pj", name="wps")
                for i in range(5):
                    nc.tensor.matmul(
                        wps[0:2, 0, :], wrm[:, 0:2], wrm[:],
                        start=(i == 0), stop=(i == 4), perf_mode=None,
                    )
                # ---- K^T projection: kt[e on partitions, k] ----
                for s in range(4):
                    for ep in range(4):  # ec pair = (2ep, 2ep+1)
                        ps = pj.tile([P, 2, 512], f32, tag="pj", name="pj")
                        # di-outer on slab 0 so the first matmuls need only
                        # the do 0:4 halves of x/wk (they land first)
                        order = (
                            [(di, half) for di in range(4) for half in range(2)]
                            if s == 0
                            else [(di, half) for half in range(2) for di in range(4)]
                        )
                        for di, half in order:
                            ec = 2 * ep + half
                            do = 2 * di
                            nc.tensor.matmul(
                                ps[:, half, :],
                                wk_sb[:, do : do + 2, ec * P : (ec + 1) * P],
                                xsb[s][:, do : do + 2, :],
                                start=(di == 0),
                                stop=(di == 3),
                                perf_mode=DR,
                            )
                        copy_out(kt_slabs[s][:, 2 * ep : 2 * ep + 2, :], ps[:])

                # ---- V projection: v[k on partitions, e] ----
                for s in range(4):
                    for kq in range(4):
                        ps = pj.tile([P, 2, 512], f32, tag="pj", name="pj")
                        for es in range(2):
                            for di in range(4):
                                do = 2 * di
                                nc.tensor.matmul(
                                    ps[:, es, :],
                                    xsb[s][:, do : do + 2, kq * P : (kq + 1) * P],
                                    wv_sb[:, do : do + 2, es * 512 : (es + 1) * 512],
                                    start=(di == 0),
                                    stop=(di == 3),
                                    perf_mode=DR,
                                )
                        copy_out(v_slabs[s][:, kq, :, :], ps[:])

                # ---- Q^T projection: qt[e on partitions, q] ----
                for ec in range(8):
                    ps = pj.tile([P, 2, 512], f32, tag="pj", name="pj")
                    for qs in range(2):
                        for di in range(4):
                            do = 2 * di
                            nc.tensor.matmul(
                                ps[:, qs, :],
                                wq_sb[:, do : do + 2, ec * P : (ec + 1) * P],
                                xq_sb[:, do : do + 2, qs * 512 : (qs + 1) * 512],
                                start=(di == 0),
                                stop=(di == 3),
                                perf_mode=DR,
                            )
                    copy_out(qt8_sb[:, ec, :, :], ps[:])

                # ---- side path: fp8-residual chains -> bf16 tiles ----
                for tgt, whi_sb, wlo_sb, ahi_sb, alo_sb in (
                    (ktb, wk_sb, wkl_sb, xs_sb, xsl_sb),
                    (qtb, wq_sb, wql_sb, xqs_sb, xqsl_sb),
                ):
                    for ep in range(4):
                        psb = pjs.tile([P, 2, SW], f32, tag="pjs", name="psb")
                        for half in range(2):
                            ec = 2 * ep + half
                            chains = (
                                (whi_sb, ahi_sb), (wlo_sb, ahi_sb),
                                (whi_sb, alo_sb),
                            )
                            for ci, (wt, at) in enumerate(chains):
                                for di in range(4):
                                    do = 2 * di
                                    nc.tensor.matmul(
                                        psb[:, half, :],
                                        wt[:, do : do + 2, ec * P : (ec + 1) * P],
                                        at[:, do : do + 2, :],
                                        start=(ci == 0 and di == 0),
                                        stop=(ci == 2 and di == 3),
                                        perf_mode=DR,
                                    )
                        copy_out(tgt[:, 2 * ep : 2 * ep + 2, :], psb[:])


                ps = pj.tile([P, 2, 512], f32, tag="pj", name="pj")
                for es in range(2):  # vb[k 0:SW, e]
                    chains = ((xs_sb, wv_sb), (xsl_sb, wv_sb), (xs_sb, wvl_sb))
                    for ci, (at, wt) in enumerate(chains):
                        for di in range(4):
                            do = 2 * di
                            nc.tensor.matmul(
                                ps[0:SW, es, :],
                                at[:, do : do + 2, :],
                                wt[:, do : do + 2, es * 512 : (es + 1) * 512],
                                start=(ci == 0 and di == 0),
                                stop=(ci == 2 and di == 3),
                                perf_mode=DR,
                            )
                copy_out(vb[0:SW, :, :], ps[0:SW, :, :])

                # ---- attention: swap PSUM pool layout ----
                pjs.release()
                pj.release()
                ps_pool = tc.alloc_tile_pool(name="ps", bufs=3, space="PSUM")
                po_pool = tc.alloc_tile_pool(name="po", bufs=2, space="PSUM")
                pd_pool = tc.alloc_tile_pool(name="pd", bufs=1, space="PSUM")

                # side scores: chunk0 keys x qc0 cols of block 0, bf16
                sps = ps_pool.tile([P, QT], f32, tag="ps", name="sps")
                for ec in range(8):
                    nc.tensor.matmul(
                        sps[0:SW, 0:SW],
                        ktb[:, ec, :],
                        qtb[:, ec, :],
                        start=(ec == 0),
                        stop=(ec == 7),
                    )
                nc.vector.tensor_add(
                    out=sps[0:SW, 0:SW], in0=sps[0:SW, 0:SW],
                    in1=mask_sb[0:SW, 0:SW],
                )
                nc.scalar.activation(
                    out=ptb[0:SW, 0:SW], in_=sps[0:SW, 0:SW],
                    func=EXP, scale=SC8,
                )

                def emit_av(q, pt, last_tile=False):
                    for qc in range(2):
                        nk = 4 * q + 2 if qc == 0 else 4 * q + 4
                        npair = nk // 2
                        blk0 = q == 0 and qc == 0
                        dn = pd_pool.tile([P, 1], f32, tag="dn")
                        pos = po_pool.tile(
                            [P, 2, 512], f32, tag="po", name="po"
                        )
                        # denominator chain + reciprocal first: rc is ready
                        # long before the pos chains finish, so the scale
                        # starts right at the last AV matmul
                        if blk0:
                            nc.tensor.matmul(
                                dn[:], ptb[:], onesb[:],
                                start=True, stop=False,
                            )
                        for pr in range(npair):
                            nc.tensor.matmul(
                                dn[:],
                                pt[:, 2 * pr : 2 * pr + 2, qc * P : (qc + 1) * P],
                                ones8[:],
                                start=(pr == 0 and not blk0),
                                stop=(pr == npair - 1),
                                perf_mode=DR,
                            )
                        rc = rc_pool.tile([P, 1], f32, tag="rc")
                        nc.vector.reciprocal(out=rc[:], in_=dn[:])
                        if blk0:
                            for es in range(2):
                                nc.tensor.matmul(
                                    pos[:, es, :],
                                    ptb[:],
                                    vb[:, es, :],
                                    start=True,
                                    stop=False,
                                )
                        for pr in range(npair):
                            first = pr == 0 and not blk0
                            last = pr == npair - 1
                            s2, kqp = pr // 2, (pr % 2) * 2
                            lhsT = pt[:, 2 * pr : 2 * pr + 2, qc * P : (qc + 1) * P]
                            for es in range(2):
                                nc.tensor.matmul(
                                    pos[:, es, :],
                                    lhsT,
                                    v_slabs[s2][:, kqp : kqp + 2, es, :],
                                    start=first,
                                    stop=last,
                                    perf_mode=DR,
                                )
                        q0 = q * QT + qc * P
                        ob = ob_pool.tile([P, 2, 512], bf, tag="ob", name="ob")
                        nc.vector.tensor_scalar_mul(
                            out=ob[:], in0=pos[:], scalar1=rc[:]
                        )
                        nc.sync.dma_start(out3[q0 : q0 + P, :, :], ob[:])

                for a, b in ((0, 1), (2, 3)):
                    pts = {
                        q: pt_pool.tile([P, 16, QT], f8, tag="pt", name=f"pt{q}")
                        for q in (a, b)
                    }
                    def score_mms(q, kc, dst, c0):
                        s, kq = kc // 4, kc % 4
                        qh, qo = q // 2, (q % 2) * QT
                        for di in range(4):
                            ec = 2 * di
                            nc.tensor.matmul(
                                dst,
                                kt_slabs[s][:, ec : ec + 2, kq * P : (kq + 1) * P],
                                qt8_sb[:, ec : ec + 2, qh, qo + c0 : qo + QT],
                                start=(di == 0),
                                stop=(di == 3),
                                perf_mode=DR,
                            )

                    for kcp in range(2 * b + 2):
                        kc0, kc1 = 2 * kcp, 2 * kcp + 1
                        active = [q for q in (a, b) if kc0 < 4 * q + 4]
                        # non-diag kc pairs share one single-bank [P,2,QT]
                        # psum and a single exp, halving the per-op ACT
                        # overhead on the phase-2 critical path
                        work = []  # (q, psum, kind)
                        for q in active:
                            if kc1 < 4 * q:  # both tiles non-diag
                                ps2 = ps_pool.tile(
                                    [P, 2, QT], f32, tag="ps", name="ps"
                                )
                                score_mms(q, kc0, ps2[:, 0, :], 0)
                                score_mms(q, kc1, ps2[:, 1, :], 0)
                                work.append((q, ps2, "pair"))
                            else:
                                for kc in (kc0, kc1):
                                    if kc >= 4 * q + 4:
                                        continue
                                    m = kc - 4 * q
                                    c0 = 64 * m if m >= 0 else 0
                                    ps1 = ps_pool.tile(
                                        [P, 2, QT], f32, tag="ps", name="ps"
                                    )
                                    score_mms(q, kc, ps1[:, 0, c0:QT], c0)
                                    work.append((q, ps1, kc))
                        for q, psx, kind in work:
                            if kind == "pair":
                                nc.scalar.activation(
                                    out=pts[q][:, 2 * kcp : 2 * kcp + 2, :],
                                    in_=psx[:],
                                    func=EXP,
                                    scale=SC8,
                                )
                                continue
                            kc = kind
                            m = kc - 4 * q
                            c0 = 64 * m if m >= 0 else 0
                            if m >= 0:
                                w0 = 64 * m
                                nc.vector.tensor_add(
                                    out=psx[:, 0, w0 : w0 + 64],
                                    in0=psx[:, 0, w0 : w0 + 64],
                                    in1=mask_sb[:],
                                )
                            nc.scalar.activation(
                                out=pts[q][:, kc, c0:QT],
                                in_=psx[:, 0, c0:QT],
                                func=EXP,
                                scale=SC8,
                            )
                            if q == 0 and kc == 0:
                                # side path provides keys 0:SW x cols 0:SW
                                nc.gpsimd.memset(pts[q][0:SW, 0, 0:SW], 0.0)
                            if m == 1:
                                nc.gpsimd.memset(pts[q][:, kc, 0:64], 0.0)
                            elif m == 3:
                                nc.gpsimd.memset(pts[q][:, kc, 128:192], 0.0)
                        if kc1 == 4 * a + 3:
                            # block a's chunks are complete: emit its AV now
                            # so the post-processing/stores drain early
                            emit_av(a, pts[a])
                    emit_av(b, pts[b], last_tile=(b == 3))
                pd_pool.release()
                po_pool.release()
                ps_pool.release()

    nc.compile()
    return nc


def _get_nc():
    if "nc" not in _cache:
        _cache["nc"] = _build()
    return _cache["nc"]


def _host_mask_window(h: int) -> np.ndarray:
    # one additive window tile [p, jj], reused by every diagonal chunk m of
    # every query block i: for score tile at key chunk kc = 4i+m, query
    # block i, the masked/boundary region is cols j in [64m, 64m+64) and
    # keep(p, j) iff p - 2j <= h - 128m  <=>  p <= 2*jj + h  (m cancels).
    p = np.arange(P)[:, None]
    jj = np.arange(64)[None, :]
    keep = p <= (2 * jj + h)
    return np.where(keep, np.float32(0.0), np.float32(NEG)).astype(np.float32)


def make_in_maps(x, Wq, Wk, Wv):
    import ml_dtypes

    F8 = ml_dtypes.float8_e4m3

    def split8(a):
        # a fp32 -> hi/lo fp8 residual pair (same orientation)
        a = np.ascontiguousarray(np.asarray(a, dtype=np.float32))
        hi = a.astype(F8)
        lo = (a - hi.astype(np.float32)).astype(F8)
        return hi, lo

    wqhi, wqlo = split8(np.asarray(Wq, dtype=np.float32).T * WS)
    wkhi, wklo = split8(np.asarray(Wk, dtype=np.float32).T * WS)
    wvhi, wvlo = split8(np.asarray(Wv, dtype=np.float32).T * WS)
    masks_h = [_host_mask_window(0), _host_mask_window(1)]
    in_maps = []
    for c in range(N_CORES):
        b, h = c // 2, c % 2
        xb = np.asarray(x[b], dtype=np.float32)
        xbT = np.ascontiguousarray(xb.T)
        xqT = np.ascontiguousarray(xb[h::2, :].T)
        xshi, xslo = split8(xb[0:SW, :].T)
        xqshi, xqslo = split8(xb[h::2, :][0:SW, :].T)
        in_maps.append(
            {
                "x8": xbT.astype(F8),
                "xq8": xqT.astype(F8),
                "wq8": wqhi, "wql": wqlo,
                "wk8": wkhi, "wkl": wklo,
                "wv8": wvhi, "wvl": wvlo,
                "xs8": xshi, "xsl": xslo,
                "xqs8": xqshi, "xqsl": xqslo,
                "maskw": masks_h[h],
            }
        )
    return in_maps


def kernel(x, Wq, Wk, Wv):
    from concourse.bass_utils import run_bass_kernel_spmd

    nc = _get_nc()
    in_maps = make_in_maps(x, Wq, Wk, Wv)
    res = run_bass_kernel_spmd(nc, in_maps, core_ids=list(range(N_CORES)))
    out = np.empty((B, S, D), dtype=np.float32)
    for c in range(N_CORES):
        b, h = c // 2, c % 2
        out[b, h::2, :] = res.results[c]["out"].astype(np.float32)
    return out
